# revision 54
# baseline (speedup 1.0000x reference)
"""Distributed causal multi-head attention (Bass/Tile, 8 TRN2 NeuronCores).

Sharding: core c owns heads (2c, 2c+1) of BOTH batches, and owns output
rows [c*256, (c+1)*256) of each batch.  QKV for its 2 heads is computed
from the full x (both batches) locally -- no K/V collective.  After each
batch's attention, one 8-rank AllToAll redistributes UNNORMALIZED
attention outputs plus the softmax denominator row (ones-column trick)
to the query-row owners; each core then normalizes (batched fast
reciprocal + PE broadcast) and applies the full Wo to its rows.

Per core, per batch b:
  q^T, k^T = (x_b @ Wq/Wk)^T  [128, 2048]  (partitions = 2 heads x 64 dims)
  v        =  x_b @ Wv        [128, 65] per (head, kchunk), ones col appended
  scores^T = k^T.T @ q^T      head-interleaved pairs run concurrently on PE
  softmax: full-width exp (scalar engine), causal mask as 0/1 post-mult,
  AV matmul with ones column -> unnormalized out^T + denominator row,
  AllToAll (bf16) across all 8 cores, normalize after gather, y = out^T.T @ Wo.

All projection / v / Wo matmul work is chopped into ~1-2us units and
drained into the attention loop between exp-paced iterations so the PE
stays dense (HAM clock-gate stays warm).
"""

import numpy as np

B, T, C, H = 2, 2048, 1024, 16
D = C // H            # 64
NQG = T // 512        # 4 query groups of 512
NKC = T // 128        # 16 key chunks of 128
CC = C // 128         # 8 contraction chunks
WCOL = 3 * 2 * D      # 384 packed qkv columns per core (2 heads)
SCALE = 1.0 / 32.0    # 1/sqrt(C), folded into Wq on host
OWN = 256             # query rows owned per (core, batch)
SH = 130 * OWN        # AllToAll shard elems: 2 heads x (64 d + den) x 256 q
HOFF = 65 * OWN       # head hh=1 offset inside a shard
SH2 = 130 * 128       # batch-1 half-exchange shard (ownership interleaved)
HOFF2 = 65 * 128

_cached_nc = None
last_result = None


def _xcol(b, cc, t):
    """xt_sb column of (batch, chunk, seq-pos) in (b, half, cc) block order."""
    return (b * 2 + t // 1024) * (CC * 1024) + cc * 1024 + (t % 1024)


def _loads(nc, P, mybir):
    from concourse.bass import ts

    F32, BF16 = mybir.dt.float32, mybir.dt.bfloat16
    AFT = mybir.ActivationFunctionType

    # small consts first on sync
    mask = P["const_p"].tile([128, 128], BF16, tag="mask")
    nc.sync.dma_start(mask[:], P["mask_ext"][:])
    P["mask"] = mask
    sel = P["const_p"].tile([16, 1024], BF16, tag="sel", name="sel")
    nc.sync.dma_start(sel[:], P["sel_ext"][:])
    P["sel"] = sel

    # weights and x^T are HOST-PACKED into their exact SBUF layouts, so
    # every load is one big contiguous 2D DMA (per-DMA trigger costs
    # ~600ns of issuing-engine time; 3D APs fall off the DMA fast path)
    wqkv_sb = P["w_p"].tile([128, CC * WCOL], BF16, tag="wqkv")
    nc.scalar.dma_start(wqkv_sb[:], P["wqkv_ext"][:])
    P["wqkv_sb"] = wqkv_sb

    # x^T in (batch, column-half, chunk) block order; batch-0 half 0 first.
    # The first block is split so the lead-in q/k accumulation chains can
    # start on early chunks while later ones are still in flight.
    xt_sb = P["x_p"].tile([128, 2 * CC * T], BF16, tag="xt")
    P["xt_sb"] = xt_sb
    cuts = [0, 2048, 4096, 6144, 8192, 12288, 16384, 24576, 32768]
    for i, (lo, hi) in enumerate(zip(cuts[:-1], cuts[1:])):
        # alternate the first block's quarters across sync+scalar so the
        # lead-in q/k chains start on early chunks ~3us sooner
        q = nc.scalar if i in (1, 3) else nc.sync
        q.dma_start(xt_sb[:, lo:hi], P["xt_ext"][:, lo:hi])

    # warm the ACT exp table while DMAs stream
    scr = P["const_p"].tile([128, 32], F32, tag="scr", name="scr")
    nc.vector.memset(scr[:, 0:16], 0.0)
    nc.scalar.activation(scr[:, 16:32], scr[:, 0:16], AFT.Exp)

    # Wo on scalar behind wqkv (needed only ~60us in)
    wo_sb = P["w_p"].tile([128, CC * C], BF16, tag="wo")
    nc.scalar.dma_start(wo_sb[:], P["wo_ext"][:])
    P["wo_sb"] = wo_sb

    # ones column of v (softmax denominator accumulator)
    for b in range(2):
        v_sb = P["v_p"].tile([128, 2 * NKC * 65], BF16, tag=f"v{b}", name=f"v{b}")
        nc.vector.memset(
            v_sb[:].rearrange("p (hj x) -> p hj x", x=65)[:, :, 64:65], 1.0
        )
        P[f"v{b}"] = v_sb


def _v_unit(nc, P, b, js, mybir):
    """v rows for batch b, key chunks js: [128, 65] per (head, kchunk)."""
    F32 = mybir.dt.float32
    xt_sb, wqkv_sb, v_sb = P["xt_sb"], P["wqkv_sb"], P[f"v{b}"]
    for j in js:
        ps = P["aux_p"].tile([128, 512], F32, tag="aux", name="vps")
        for cc in range(CC):
            x0 = _xcol(b, cc, j * 128)
            nc.tensor.matmul(
                ps[:, 0:128],
                xt_sb[:, x0 : x0 + 128],
                wqkv_sb[:, cc * WCOL + 256 : cc * WCOL + 384],
                start=(cc == 0),
                stop=(cc == CC - 1),
            )
        nc.vector.tensor_copy(
            v_sb[:].rearrange("p (hj x) -> p hj x", x=65)[:, j::NKC, 0:64],
            ps[:, 0:128].rearrange("p (h d) -> p h d", d=64),
        )


def _qk_unit(nc, P, b, kind, tb, nh, mybir):
    """One 512-col block of q^T or k^T for batch b (8 matmuls + 1 copy)."""
    F32 = mybir.dt.float32
    xt_sb, wqkv_sb = P["xt_sb"], P["wqkv_sb"]
    dst = P["qt"][b] if kind == 0 else P["kt"][b]
    mcol = kind * 128
    t0 = tb * 1024 + nh * 512
    ps = P["aux_p"].tile([128, 512], F32, tag="aux", name="qkps")
    for cc in range(CC):
        x0 = _xcol(b, cc, t0)
        nc.tensor.matmul(
            ps[:],
            wqkv_sb[:, cc * WCOL + mcol : cc * WCOL + mcol + 128],
            xt_sb[:, x0 : x0 + 512],
            start=(cc == 0),
            stop=(cc == CC - 1),
        )
    nc.vector.tensor_copy(dst[:, t0 : t0 + 512], ps[:])


def _gath_reads(nc, P, key, mybir):
    """Read pair-block columns of an AllToAll output into SBUF.

    All reads ride the gpsimd queue directly behind their A2A
    instruction: that queue is blocked on the collective anyway, so the
    reads fire the moment it completes -- the scheduler cannot push them
    behind later bounce writes as it happens on the sync queue.
    """
    gath = P[f"gath{key}"]
    den = P[f"den{key}"]
    gat = P[f"gat{key}"]
    gv = gath.rearrange("(s p f) -> p s f", s=8, p=130)
    # dens: row 64 (hh=0) of each shard -> den rows 0:8, row 129 -> rows 8:16
    nc.gpsimd.dma_start(
        den[0:8, :], gv[64:65, :, :].rearrange("p s f -> (p s) f")
    )
    nc.gpsimd.dma_start(
        den[8:16, :], gv[129:130, :, :].rearrange("p s f -> (p s) f")
    )
    # d-rows: heads 2s at partitions 0:64, heads 2s+1 at 64:128
    nc.gpsimd.dma_start(
        gat[0:64, :].rearrange("p (s f) -> p s f", s=8), gv[0:64, :, :]
    )
    nc.gpsimd.dma_start(
        gat[64:128, :].rearrange("p (s f) -> p s f", s=8), gv[65:129, :, :]
    )


def _recip_unit(nc, P, key, W, gate, mybir):
    F32 = mybir.dt.float32
    # The Tile scheduler orders engine queues by data dependencies, NOT
    # emission order: an ungated den read gets hoisted to the front of the
    # DVE queue and freezes everything behind it until the AllToAll lands.
    # Gate the first den touch on a tile produced by the work that must
    # run first.
    tmp = P["sm_p"].tile([16, W], F32, tag="smf", name=f"gate{key}")
    nc.vector.tensor_scalar_mul(tmp[:], gate[0:16, 0:W], 0.0)
    denf = P["sm_p"].tile([16, W], F32, tag="smf", name=f"denf{key}")
    nc.vector.tensor_add(denf[:], P[f"den{key}"][:], tmp[:])
    rec = P["sm_p"].tile([16, W], F32, tag="smf", name=f"rec{key}")
    nc.vector.reciprocal_approx_fast(out=rec[:], in_=denf[:])
    recb = P["sm_p"].tile(
        [16, W], mybir.dt.bfloat16, tag=f"recb{key}", name=f"recb{key}"
    )
    nc.vector.tensor_copy(recb[:], rec[:])
    P[f"recb{key}"] = recb


def _bc_mult_unit(nc, P, key, W, mybir):
    """Normalize gat{key} in place: PE broadcast of 1/den + DVE mult."""
    F32 = mybir.dt.float32
    gat, sel, recb = P[f"gat{key}"], P["sel"], P[f"recb{key}"]
    for s in range(8):
        bc = P["aux_p"].tile([128, 512], F32, tag="aux", name="bc")
        nc.tensor.matmul(
            bc[:, 0:W], sel[:, s * 128 : (s + 1) * 128], recb[:],
            start=True, stop=True,
        )
        blk = gat[:, s * W : (s + 1) * W]
        nc.vector.tensor_mul(blk, blk, bc[:, 0:W])


def _wo_unit(nc, P, key, W, row0, thfs, mybir):
    """Wo contraction (full 1024 chan) for y blocks thfs of gat{key}."""
    F32 = mybir.dt.float32
    gat, wo_sb = P[f"gat{key}"], P["wo_sb"]
    for t, hf in thfs:
        ps = P["aux_p"].tile([128, 512], F32, tag="aux", name="wops")
        for s in range(8):
            nc.tensor.matmul(
                ps[:],
                gat[:, s * W + t * 128 : s * W + (t + 1) * 128],
                wo_sb[:, s * C + hf * 512 : s * C + (hf + 1) * 512],
                start=(s == 0),
                stop=(s == 7),
            )
        yb = P["y_p"].tile([128, 512], F32, tag="y", name="yb")
        nc.vector.tensor_copy(yb[:], ps[:])
        P["last_yb"] = yb
        nc.sync.dma_start(
            P["out_ext"][
                row0 + t * 128 : row0 + (t + 1) * 128,
                hf * 512 : (hf + 1) * 512,
            ],
            yb[:],
        )


def _attention_qg(nc, P, b, qg, fillers, mybir):
    """Scores^T + exp + AV for batch b's two heads, one query group.

    fillers: list of zero-arg closures emitting background PE work; one is
    drained per jp iteration (after scores/exp, before the pipelined AV),
    leftovers at the end of the group.
    """
    F32, BF16 = mybir.dt.float32, mybir.dt.bfloat16
    AFT = mybir.ActivationFunctionType
    qt, kt, v_sb, mask = P["qt"][b], P["kt"][b], P[f"v{b}"], P["mask"]

    njc = 4 * qg + 4          # key chunks (incl. diagonal) for this block
    avs = [
        P["av_p"].tile([65, 512], F32, tag="av", name=f"av{hh}")
        for hh in range(2)
    ]
    # masks: all batch-1 masks on vector -- the gpsimd queue hosts the A2A0
    # collective instruction, which blocks it until the collective completes
    meng = nc.gpsimd if (b == 0 and qg >= 2) else nc.vector

    def emit_avs(att2, jp):
        for hh in range(2):
            for dj in range(2):
                j = 2 * jp + dj
                lo = max((j - 4 * qg) * 128, 0)
                nc.tensor.matmul(
                    avs[hh][:, lo:],
                    v_sb[:, (hh * NKC + j) * 65 : (hh * NKC + j) * 65 + 65],
                    att2[hh][:, dj * 512 + lo : (dj + 1) * 512],
                    start=(j == 0),
                    stop=(j == njc - 1),
                )

    pend = None  # 1-deep software pipeline: scores(jp+1) before AV(jp)
    for jp in range(njc // 2):
        # interleave the two heads' score matmuls so the (0,0)/(64,0) PE
        # tiles run concurrently
        ps2 = [
            P["mm_p"].tile([128, 1024], F32, tag="mm", name=f"scps{hh}")
            for hh in range(2)
        ]
        for dj in range(2):
            j = 2 * jp + dj
            lo = max((j - 4 * qg) * 128, 0)  # skip sub-causal columns
            for hh in range(2):
                nc.tensor.matmul(
                    ps2[hh][:, dj * 512 + lo : (dj + 1) * 512],
                    kt[hh * 64 : (hh + 1) * 64, j * 128 : (j + 1) * 128],
                    qt[hh * 64 : (hh + 1) * 64, qg * 512 + lo : (qg + 1) * 512],
                    start=True,
                    stop=True,
                    tile_position=(hh * 64, 0),
                )
        st = 256 if jp == njc // 2 - 1 else 0  # last jp: cols<256 sub-causal
        att2 = []
        for hh in range(2):
            a2 = P["att_p"].tile([128, 1024], BF16, tag="att", name="a2")
            nc.scalar.activation(a2[:, st:], ps2[hh][:, st:], AFT.Exp)
            att2.append(a2)
        if b == 1 and qg == 3 and jp == 2:
            # gate object for batch-0's normalize chain: anything mid-qg3
            P["gate_att"] = att2[0]
        for dj in range(2):
            j = 2 * jp + dj
            l0 = (j - 4 * qg) * 128
            if l0 >= 0:  # diagonal chunk: triangular 0/1 mask
                for hh in range(2):
                    meng.tensor_mul(
                        att2[hh][:, dj * 512 + l0 : dj * 512 + l0 + 128],
                        att2[hh][:, dj * 512 + l0 : dj * 512 + l0 + 128],
                        mask[:],
                    )
        if fillers and (len(fillers) > 1 or jp < njc // 2 - 1):
            fillers.pop(0)()
        if pend is not None:
            emit_avs(*pend)
        pend = (att2, jp)
    while len(fillers) > 1:
        fillers.pop(0)()
    emit_avs(*pend)
    # one filler held back to cover the PE while the final AV chain and
    # the avs evacuation drain (kills the ~2.4us qg-boundary bubble)
    for f in fillers:
        f()
    fillers.clear()

    # evacuate unnormalized out^T + den row straight to the bounce shards.
    # batch 0: the 512-query group spans owners 2qg, 2qg+1 (256 q each);
    # batch 1: ownership is interleaved at 128-q granularity across the
    # two half-exchanges (bounce1a = qg0/1, bounce1b = qg2/3) so the
    # first half's AllToAll fires mid-attention
    for hh in range(2):
        ob = P["ob_p"].tile([65, 512], BF16, tag="ob", name="ob")
        nc.vector.tensor_copy(ob[:], avs[hh][:])
        P["last_ob"] = ob
        if b == 0:
            bnc = P["bounce0"]
            for half in range(2):
                sh = 2 * qg + half
                nc.sync.dma_start(
                    bnc[sh * SH + hh * HOFF : sh * SH + hh * HOFF + HOFF]
                    .rearrange("(q f) -> q f", q=65),
                    ob[:, half * 256 : (half + 1) * 256],
                )
        else:
            bnc = P["bounce1a" if qg < 2 else "bounce1b"]
            for c in range(4):
                sh = (qg % 2) * 4 + c
                nc.sync.dma_start(
                    bnc[sh * SH2 + hh * HOFF2 : sh * SH2 + hh * HOFF2 + HOFF2]
                    .rearrange("(q f) -> q f", q=65),
                    ob[:, c * 128 : (c + 1) * 128],
                )


def _heartbeat(nc, P, steps, mybir):
    """Dep-chained dummy-matmul ladder: keeps the PE's HAM clock warm
    across a collective wait.  Each step is 4 N=512 matmuls (~75% PE
    duty) gated on a DVE copy of the previous step's result."""
    F32, BF16 = mybir.dt.float32, mybir.dt.bfloat16
    hb = [
        P["const_p"].tile([16, 16], BF16, tag=f"hb{i}", name=f"hb{i}")
        for i in range(2)
    ]
    nc.vector.memset(hb[0][:], 0.25)
    for k in range(steps):
        ps = P["mm_p"].tile([128, 1024], F32, tag="mm", name="hbps")
        nc.tensor.matmul(
            ps[0:16, 0:16], hb[k % 2][:], hb[k % 2][:], start=True, stop=True
        )
        # chain via the scalar engine (idle in the tail): a DVE-side chain
        # could be frozen by a hoisted collective-dependent DVE wait
        nc.scalar.copy(hb[(k + 1) % 2][:], ps[0:16, 0:16])


def _a2a(nc, P, key, mybir):
    """AllToAll the bounced shards of one exchange across all 8 cores."""
    groups = [[0, 1, 2, 3, 4, 5, 6, 7]]
    nc.gpsimd.collective_compute(
        "AllToAll", mybir.AluOpType.bypass, replica_groups=groups,
        ins=[P[f"bounce{key}"].opt()], outs=[P[f"gath{key}"].opt()],
    )


def _body(nc, P, mybir):
    F32, BF16 = mybir.dt.float32, mybir.dt.bfloat16
    _loads(nc, P, mybir)

    for b in range(2):
        qt_t = P["qk_p"].tile([128, T], BF16, tag=f"qt{b}", name=f"qt{b}")
        kt_t = P["qk_p"].tile([128, T], BF16, tag=f"kt{b}", name=f"kt{b}")
        P.setdefault("qt", []).append(qt_t)
        P.setdefault("kt", []).append(kt_t)
    # batch 0: one full exchange; batch 1: two half exchanges (1a, 1b)
    for key, W, shsz in [("0", OWN, SH), ("1a", 128, SH2), ("1b", 128, SH2)]:
        P[f"gat{key}"] = P["gat_p"].tile(
            [128, 8 * W], BF16, tag=f"gat{key}", name=f"gat{key}"
        )
        P[f"den{key}"] = P["sm_p"].tile(
            [16, W], BF16, tag=f"den{key}", name=f"den{key}"
        )
        P[f"bounce{key}"] = P["dram_p"].tile(
            [8 * shsz], BF16, tag=f"bounce{key}", name=f"bounce{key}"
        )
        P[f"gath{key}"] = P["dram_p"].tile(
            [8 * shsz], BF16, tag=f"gath{key}", name=f"gath{key}"
        )

    U = lambda *a: (lambda: _qk_unit(nc, P, *a, mybir))
    V = lambda b, *js: (lambda: _v_unit(nc, P, b, js, mybir))
    WO = lambda b, *thfs: (lambda: _wo_unit(nc, P, b, thfs, mybir))

    # lead-in: just enough q^T/k^T/v for batch-0 qg0
    _qk_unit(nc, P, 0, 1, 0, 0, mybir)
    _qk_unit(nc, P, 0, 0, 0, 0, mybir)
    _v_unit(nc, P, 0, [0, 1], mybir)

    _attention_qg(nc, P, 0, 0, [V(0, 2, 3), U(0, 1, 0, 1), U(0, 0, 0, 1)], mybir)
    _attention_qg(
        nc, P, 0, 1, [V(0, 4, 5), V(0, 6, 7), U(0, 1, 1, 0), U(0, 0, 1, 0)], mybir
    )
    _attention_qg(
        nc, P, 0, 2,
        [V(0, 8, 9), V(0, 10, 11), U(0, 1, 1, 1), U(0, 0, 1, 1),
         U(1, 1, 0, 0), U(1, 0, 0, 0)],
        mybir,
    )
    _attention_qg(
        nc, P, 0, 3,
        [V(0, 12, 13), V(0, 14, 15), V(1, 0, 1), V(1, 2, 3),
         U(1, 1, 0, 1), U(1, 0, 0, 1), U(1, 1, 1, 0), U(1, 0, 1, 0)],
        mybir,
    )
    _a2a(nc, P, "0", mybir)
    _gath_reads(nc, P, "0", mybir)

    _attention_qg(nc, P, 1, 0, [V(1, 4, 5), V(1, 6, 7)], mybir)
    _attention_qg(
        nc, P, 1, 1,
        [V(1, 8, 9), V(1, 10, 11), U(1, 1, 1, 1), U(1, 0, 1, 1)],
        mybir,
    )
    # first half-exchange fires mid-attention, fully hidden under qg2/3
    _a2a(nc, P, "1a", mybir)
    _gath_reads(nc, P, "1a", mybir)
    _attention_qg(nc, P, 1, 2, [V(1, 12, 13)], mybir)
    _attention_qg(nc, P, 1, 3, [V(1, 14, 15)], mybir)
    _a2a(nc, P, "1b", mybir)
    _gath_reads(nc, P, "1b", mybir)

    # batch-0's Wo + the first-half batch-1 Wo fill the A2A1b wait (their
    # collectives completed long ago); all normalize chains gate on qg3's
    # last evacuation so they cannot freeze the attention DVE stream
    _recip_unit(nc, P, "0", 256, P["last_ob"], mybir)
    _bc_mult_unit(nc, P, "0", 256, mybir)
    for t in range(2):
        _wo_unit(nc, P, "0", 256, 0, [(t, 0), (t, 1)], mybir)
    # each chain gates on the previous chain's output so the DVE queue
    # serves them in order (1a's data can lag the qg3 finish on slow draws)
    _recip_unit(nc, P, "1a", 128, P["last_yb"], mybir)
    _bc_mult_unit(nc, P, "1a", 128, mybir)
    _wo_unit(nc, P, "1a", 128, 256, [(0, 0), (0, 1)], mybir)
    _heartbeat(nc, P, 10, mybir)
    # tail: only the half-size A2A1b + half a Wo pass remain exposed
    _recip_unit(nc, P, "1b", 128, P["last_yb"], mybir)
    _bc_mult_unit(nc, P, "1b", 128, mybir)
    _wo_unit(nc, P, "1b", 128, 384, [(0, 0), (0, 1)], mybir)


def _build():
    import concourse.mybir as mybir
    import concourse.tile as tile
    from concourse import bacc

    F32, BF16 = mybir.dt.float32, mybir.dt.bfloat16

    nc = bacc.Bacc("TRN2", target_bir_lowering=False, debug=False, num_devices=8)
    P = {
        "xt_ext": nc.declare_dram_parameter(
            "xt", [128, 2 * CC * T], BF16, isOutput=False
        ),
        "wqkv_ext": nc.declare_dram_parameter(
            "wqkv", [128, CC * WCOL], BF16, isOutput=False
        ),
        "wo_ext": nc.declare_dram_parameter(
            "wo", [128, CC * C], BF16, isOutput=False
        ),
        "mask_ext": nc.declare_dram_parameter("mask", [128, 128], BF16, isOutput=False),
        "sel_ext": nc.declare_dram_parameter("sel", [16, 1024], BF16, isOutput=False),
        "out_ext": nc.declare_dram_parameter("out", [2 * OWN, C], F32, isOutput=True),
    }

    with tile.TileContext(nc) as tc:
        with (
            tc.tile_pool(name="const", bufs=1) as const_p,
            tc.tile_pool(name="w", bufs=1) as w_p,
            tc.tile_pool(name="x", bufs=1) as x_p,
            tc.tile_pool(name="qk", bufs=1) as qk_p,
            tc.tile_pool(name="v", bufs=1) as v_p,
            tc.tile_pool(name="att", bufs=4) as att_p,
            tc.tile_pool(name="ob", bufs=4) as ob_p,
            tc.tile_pool(name="gat", bufs=1) as gat_p,
            tc.tile_pool(name="y", bufs=2) as y_p,
            tc.tile_pool(name="sm", bufs=2) as sm_p,
            tc.tile_pool(name="mm", bufs=2, space="PSUM") as mm_p,
            tc.tile_pool(name="av", bufs=2, space="PSUM") as av_p,
            tc.tile_pool(name="aux", bufs=2, space="PSUM") as aux_p,
            tc.tile_pool(name="dram", bufs=1, space="DRAM") as dram_p,
        ):
            P.update(
                const_p=const_p, w_p=w_p, x_p=x_p, qk_p=qk_p, v_p=v_p,
                att_p=att_p, ob_p=ob_p, gat_p=gat_p, y_p=y_p, sm_p=sm_p,
                mm_p=mm_p, av_p=av_p, aux_p=aux_p, dram_p=dram_p,
            )
            _body(nc, P, mybir)

    nc.finalize()
    return nc


def kernel(x, Wqkv, bqkv, Wo, bo):
    global _cached_nc, last_result
    import ml_dtypes
    from concourse.bass_utils import run_bass_kernel_spmd

    if _cached_nc is None:
        _cached_nc = _build()
    nc = _cached_nc

    bf16 = ml_dtypes.bfloat16
    x = np.asarray(x, dtype=np.float32)
    Wqkv = np.asarray(Wqkv, dtype=np.float32)
    # Wo packed into its SBUF layout: [128, cc-chunks of 1024]
    wo_f = np.asarray(Wo, dtype=np.float32)
    wo_b = np.ascontiguousarray(
        np.concatenate(
            [wo_f[cc * 128 : (cc + 1) * 128, :] for cc in range(CC)], axis=1
        ).astype(bf16)
    )

    # x^T packed into its SBUF layout: [128, (batch, col-half, chunk) x 1024]
    xt_blocks = []
    for b in range(2):
        xb = x[b].T.astype(bf16)  # [C, T]
        for th in range(2):
            for cc in range(CC):
                xt_blocks.append(
                    xb[cc * 128 : (cc + 1) * 128, th * 1024 : (th + 1) * 1024]
                )
    xt = np.ascontiguousarray(np.concatenate(xt_blocks, axis=1))

    # lower-triangle 0/1 mask for diagonal blocks: partition = key, free = query
    tri = (np.arange(128)[:, None] <= np.arange(128)[None, :]).astype(bf16)
    tri = np.ascontiguousarray(tri)

    # selection matrix for the 1/den PE broadcast: chunk s rows 0:64 get
    # head 2s's den (sel row s), rows 64:128 get head 2s+1's (row 8+s)
    sel = np.zeros((16, 1024), dtype=bf16)
    for s in range(8):
        sel[s, s * 128 : s * 128 + 64] = 1
        sel[8 + s, s * 128 + 64 : s * 128 + 128] = 1
    sel = np.ascontiguousarray(sel)

    in_maps = []
    for core in range(8):
        c0 = core * 2 * D
        wq = Wqkv[:, c0 : c0 + 128] * SCALE
        wk = Wqkv[:, C + c0 : C + c0 + 128]
        wv = Wqkv[:, 2 * C + c0 : 2 * C + c0 + 128]
        wfull = np.concatenate([wq, wk, wv], axis=1).astype(bf16)  # [C, 384]
        # packed into SBUF layout: [128, cc-chunks of 384]
        wqkv = np.ascontiguousarray(
            np.concatenate(
                [wfull[cc * 128 : (cc + 1) * 128, :] for cc in range(CC)],
                axis=1,
            )
        )
        in_maps.append(
            {"xt": xt, "wqkv": wqkv, "wo": wo_b, "mask": tri, "sel": sel}
        )

    last_result = run_bass_kernel_spmd(nc, in_maps, core_ids=list(range(8)))

    y = np.empty((B, T, C), dtype=np.float32)
    for core in range(8):
        r = last_result.results[core]["out"]
        y[0, core * OWN : (core + 1) * OWN, :] = r[0:OWN]
        # batch 1 ownership is interleaved at 128 rows per half-exchange
        y[1, core * 128 : (core + 1) * 128, :] = r[256:384]
        y[1, 1024 + core * 128 : 1024 + (core + 1) * 128, :] = r[384:512]
    return y


# revision 56
# speedup vs baseline: 1.0801x; 1.0801x over previous
"""Distributed causal multi-head attention (Bass/Tile, 8 TRN2 NeuronCores).

Sharding: core c owns heads (2c, 2c+1) of BOTH batches, and owns output
rows [c*256, (c+1)*256) of each batch.  QKV for its 2 heads is computed
from the full x (both batches) locally -- no K/V collective.  After each
batch's attention, one 8-rank AllToAll redistributes UNNORMALIZED
attention outputs plus the softmax denominator row (ones-column trick)
to the query-row owners; each core then normalizes (batched fast
reciprocal + PE broadcast) and applies the full Wo to its rows.

Per core, per batch b:
  q^T, k^T = (x_b @ Wq/Wk)^T  [128, 2048]  (partitions = 2 heads x 64 dims)
  v        =  x_b @ Wv        [128, 65] per (head, kchunk), ones col appended
  scores^T = k^T.T @ q^T      head-interleaved pairs run concurrently on PE
  softmax: full-width exp (scalar engine), causal mask as 0/1 post-mult,
  AV matmul with ones column -> unnormalized out^T + denominator row,
  AllToAll (bf16) across all 8 cores, normalize after gather, y = out^T.T @ Wo.

All projection / v / Wo matmul work is chopped into ~1-2us units and
drained into the attention loop between exp-paced iterations so the PE
stays dense (HAM clock-gate stays warm).
"""

import numpy as np

B, T, C, H = 2, 2048, 1024, 16
D = C // H            # 64
NQG = T // 512        # 4 query groups of 512
NKC = T // 128        # 16 key chunks of 128
CC = C // 128         # 8 contraction chunks
WCOL = 3 * 2 * D      # 384 packed qkv columns per core (2 heads)
SCALE = 1.0 / 32.0    # 1/sqrt(C), folded into Wq on host
OWN = 256             # query rows owned per (core, batch)
SH = 130 * OWN        # AllToAll shard elems: 2 heads x (64 d + den) x 256 q
HOFF = 65 * OWN       # head hh=1 offset inside a shard
SH2 = 130 * 128       # batch-1 half-exchange shard (ownership interleaved)
HOFF2 = 65 * 128

_cached_nc = None
last_result = None


def _xcol(b, cc, t):
    """xt_sb column of (batch, chunk, seq-pos) in (b, half, cc) block order."""
    return (b * 2 + t // 1024) * (CC * 1024) + cc * 1024 + (t % 1024)


def _loads(nc, P, mybir):
    from concourse.bass import ts

    F32, BF16 = mybir.dt.float32, mybir.dt.bfloat16
    AFT = mybir.ActivationFunctionType

    # small consts first on sync
    mask = P["const_p"].tile([128, 128], BF16, tag="mask")
    nc.sync.dma_start(mask[:], P["mask_ext"][:])
    P["mask"] = mask
    sel = P["const_p"].tile([16, 1024], BF16, tag="sel", name="sel")
    nc.sync.dma_start(sel[:], P["sel_ext"][:])
    P["sel"] = sel

    # weights and x^T are HOST-PACKED into their exact SBUF layouts, so
    # every load is one big contiguous 2D DMA (per-DMA trigger costs
    # ~600ns of issuing-engine time; 3D APs fall off the DMA fast path)
    wqkv_sb = P["w_p"].tile([128, CC * WCOL], BF16, tag="wqkv")
    nc.scalar.dma_start(wqkv_sb[:], P["wqkv_ext"][:])
    P["wqkv_sb"] = wqkv_sb

    # x^T in (batch, column-half, chunk) block order; batch-0 half 0 first.
    # The first block is split so the lead-in q/k accumulation chains can
    # start on early chunks while later ones are still in flight.
    xt_sb = P["x_p"].tile([128, 2 * CC * T], BF16, tag="xt")
    P["xt_sb"] = xt_sb
    # the critical first block (batch0, cols 0:1024) alternates across the
    # two early-usable DMA queues (sync+scalar; DMAs can only come from
    # sync/scalar/gpsimd, and gpsimd is blocked by the collective entry
    # barrier at kernel start)
    cuts = [0, 2048, 4096, 6144, 8192, 12288, 16384, 24576, 32768]
    for i, (lo, hi) in enumerate(zip(cuts[:-1], cuts[1:])):
        q = nc.scalar if i in (1, 3) else nc.sync
        q.dma_start(xt_sb[:, lo:hi], P["xt_ext"][:, lo:hi])

    # warm the ACT exp table while DMAs stream
    scr = P["const_p"].tile([128, 32], F32, tag="scr", name="scr")
    nc.vector.memset(scr[:, 0:16], 0.0)
    nc.scalar.activation(scr[:, 16:32], scr[:, 0:16], AFT.Exp)

    # Wo on scalar behind wqkv (needed only ~60us in)
    wo_sb = P["w_p"].tile([128, CC * C], BF16, tag="wo")
    nc.scalar.dma_start(wo_sb[:], P["wo_ext"][:])
    P["wo_sb"] = wo_sb

    # ones column of v (softmax denominator accumulator)
    for b in range(2):
        v_sb = P["v_p"].tile([128, 2 * NKC * 65], BF16, tag=f"v{b}", name=f"v{b}")
        nc.vector.memset(
            v_sb[:].rearrange("p (hj x) -> p hj x", x=65)[:, :, 64:65], 1.0
        )
        P[f"v{b}"] = v_sb


def _v_unit(nc, P, b, js, mybir):
    """v rows for batch b, key chunks js: [128, 65] per (head, kchunk)."""
    F32 = mybir.dt.float32
    xt_sb, wqkv_sb, v_sb = P["xt_sb"], P["wqkv_sb"], P[f"v{b}"]
    for j in js:
        ps = P["aux_p"].tile([128, 512], F32, tag="aux", name="vps")
        for cc in range(CC):
            x0 = _xcol(b, cc, j * 128)
            nc.tensor.matmul(
                ps[:, 0:128],
                xt_sb[:, x0 : x0 + 128],
                wqkv_sb[:, cc * WCOL + 256 : cc * WCOL + 384],
                start=(cc == 0),
                stop=(cc == CC - 1),
            )
        nc.vector.tensor_copy(
            v_sb[:].rearrange("p (hj x) -> p hj x", x=65)[:, j::NKC, 0:64],
            ps[:, 0:128].rearrange("p (h d) -> p h d", d=64),
        )


def _qk_unit(nc, P, b, kind, tb, nh, mybir):
    """One 512-col block of q^T or k^T for batch b (8 matmuls + 1 copy)."""
    F32 = mybir.dt.float32
    xt_sb, wqkv_sb = P["xt_sb"], P["wqkv_sb"]
    dst = P["qt"][b] if kind == 0 else P["kt"][b]
    mcol = kind * 128
    t0 = tb * 1024 + nh * 512
    ps = P["aux_p"].tile([128, 512], F32, tag="aux", name="qkps")
    for cc in range(CC):
        x0 = _xcol(b, cc, t0)
        nc.tensor.matmul(
            ps[:],
            wqkv_sb[:, cc * WCOL + mcol : cc * WCOL + mcol + 128],
            xt_sb[:, x0 : x0 + 512],
            start=(cc == 0),
            stop=(cc == CC - 1),
        )
    nc.vector.tensor_copy(dst[:, t0 : t0 + 512], ps[:])


def _gath_reads(nc, P, key, mybir):
    """Read pair-block columns of an AllToAll output into SBUF.

    All reads ride the gpsimd queue directly behind their A2A
    instruction: that queue is blocked on the collective anyway, so the
    reads fire the moment it completes -- the scheduler cannot push them
    behind later bounce writes as it happens on the sync queue.
    """
    gath = P[f"gath{key}"]
    den = P[f"den{key}"]
    gat = P[f"gat{key}"]
    gv = gath.rearrange("(s p f) -> p s f", s=8, p=130)
    # dens: row 64 (hh=0) of each shard -> den rows 0:8, row 129 -> rows 8:16
    nc.gpsimd.dma_start(
        den[0:8, :], gv[64:65, :, :].rearrange("p s f -> (p s) f")
    )
    nc.gpsimd.dma_start(
        den[8:16, :], gv[129:130, :, :].rearrange("p s f -> (p s) f")
    )
    # d-rows: heads 2s at partitions 0:64, heads 2s+1 at 64:128
    nc.gpsimd.dma_start(
        gat[0:64, :].rearrange("p (s f) -> p s f", s=8), gv[0:64, :, :]
    )
    nc.gpsimd.dma_start(
        gat[64:128, :].rearrange("p (s f) -> p s f", s=8), gv[65:129, :, :]
    )


def _recip_unit(nc, P, key, W, gate, mybir):
    F32 = mybir.dt.float32
    # The Tile scheduler orders engine queues by data dependencies, NOT
    # emission order: an ungated den read gets hoisted to the front of the
    # DVE queue and freezes everything behind it until the AllToAll lands.
    # Gate the first den touch on a tile produced by the work that must
    # run first.
    tmp = P["sm_p"].tile([16, W], F32, tag="smf", name=f"gate{key}")
    nc.vector.tensor_scalar_mul(tmp[:], gate[0:16, 0:W], 0.0)
    denf = P["sm_p"].tile([16, W], F32, tag="smf", name=f"denf{key}")
    nc.vector.tensor_add(denf[:], P[f"den{key}"][:], tmp[:])
    rec = P["sm_p"].tile([16, W], F32, tag="smf", name=f"rec{key}")
    nc.vector.reciprocal_approx_fast(out=rec[:], in_=denf[:])
    recb = P["sm_p"].tile(
        [16, W], mybir.dt.bfloat16, tag=f"recb{key}", name=f"recb{key}"
    )
    nc.vector.tensor_copy(recb[:], rec[:])
    P[f"recb{key}"] = recb


def _bc_mult_unit(nc, P, key, W, mybir):
    """Normalize gat{key} in place: PE broadcast of 1/den + DVE mult."""
    F32 = mybir.dt.float32
    gat, sel, recb = P[f"gat{key}"], P["sel"], P[f"recb{key}"]
    for s in range(8):
        bc = P["aux_p"].tile([128, 512], F32, tag="aux", name="bc")
        nc.tensor.matmul(
            bc[:, 0:W], sel[:, s * 128 : (s + 1) * 128], recb[:],
            start=True, stop=True,
        )
        blk = gat[:, s * W : (s + 1) * W]
        nc.vector.tensor_mul(blk, blk, bc[:, 0:W])


def _wo_unit(nc, P, key, W, row0, thfs, mybir):
    """Wo contraction (full 1024 chan) for y blocks thfs of gat{key}."""
    F32 = mybir.dt.float32
    gat, wo_sb = P[f"gat{key}"], P["wo_sb"]
    for t, hf in thfs:
        ps = P["aux_p"].tile([128, 512], F32, tag="aux", name="wops")
        for s in range(8):
            nc.tensor.matmul(
                ps[:],
                gat[:, s * W + t * 128 : s * W + (t + 1) * 128],
                wo_sb[:, s * C + hf * 512 : s * C + (hf + 1) * 512],
                start=(s == 0),
                stop=(s == 7),
            )
        yb = P["y_p"].tile([128, 512], F32, tag="y", name="yb")
        nc.vector.tensor_copy(yb[:], ps[:])
        P["last_yb"] = yb
        nc.sync.dma_start(
            P["out_ext"][
                row0 + t * 128 : row0 + (t + 1) * 128,
                hf * 512 : (hf + 1) * 512,
            ],
            yb[:],
        )


def _attention_qg(nc, P, b, qg, fillers, mybir):
    """Scores^T + exp + AV for batch b's two heads, one query group.

    fillers: list of zero-arg closures emitting background PE work; one is
    drained per jp iteration (after scores/exp, before the pipelined AV),
    leftovers at the end of the group.
    """
    F32, BF16 = mybir.dt.float32, mybir.dt.bfloat16
    AFT = mybir.ActivationFunctionType
    qt, kt, v_sb, mask = P["qt"][b], P["kt"][b], P[f"v{b}"], P["mask"]

    njc = 4 * qg + 4          # key chunks (incl. diagonal) for this block
    avs = [
        P["av_p"].tile([65, 512], F32, tag="av", name=f"av{hh}")
        for hh in range(2)
    ]
    # masks: all batch-1 masks on vector -- the gpsimd queue hosts the A2A0
    # collective instruction, which blocks it until the collective completes
    meng = nc.gpsimd if (b == 0 and qg >= 2) else nc.vector

    def emit_avs(att2, jp):
        for hh in range(2):
            for dj in range(2):
                j = 2 * jp + dj
                lo = max((j - 4 * qg) * 128, 0)
                nc.tensor.matmul(
                    avs[hh][:, lo:],
                    v_sb[:, (hh * NKC + j) * 65 : (hh * NKC + j) * 65 + 65],
                    att2[hh][:, dj * 512 + lo : (dj + 1) * 512],
                    start=(j == 0),
                    stop=(j == njc - 1),
                )

    pend = None  # 1-deep software pipeline: scores(jp+1) before AV(jp)
    for jp in range(njc // 2):
        # interleave the two heads' score matmuls so the (0,0)/(64,0) PE
        # tiles run concurrently
        ps2 = [
            P["mm_p"].tile([128, 1024], F32, tag="mm", name=f"scps{hh}")
            for hh in range(2)
        ]
        for dj in range(2):
            j = 2 * jp + dj
            lo = max((j - 4 * qg) * 128, 0)  # skip sub-causal columns
            for hh in range(2):
                nc.tensor.matmul(
                    ps2[hh][:, dj * 512 + lo : (dj + 1) * 512],
                    kt[hh * 64 : (hh + 1) * 64, j * 128 : (j + 1) * 128],
                    qt[hh * 64 : (hh + 1) * 64, qg * 512 + lo : (qg + 1) * 512],
                    start=True,
                    stop=True,
                    tile_position=(hh * 64, 0),
                )
        st = 256 if jp == njc // 2 - 1 else 0  # last jp: cols<256 sub-causal
        att2 = []
        for hh in range(2):
            a2 = P["att_p"].tile([128, 1024], BF16, tag="att", name="a2")
            nc.scalar.activation(a2[:, st:], ps2[hh][:, st:], AFT.Exp)
            att2.append(a2)
        if b == 1 and qg == 3 and jp == 2:
            # gate object for batch-0's normalize chain: anything mid-qg3
            P["gate_att"] = att2[0]
        for dj in range(2):
            j = 2 * jp + dj
            l0 = (j - 4 * qg) * 128
            if l0 >= 0:  # diagonal chunk: triangular 0/1 mask
                for hh in range(2):
                    meng.tensor_mul(
                        att2[hh][:, dj * 512 + l0 : dj * 512 + l0 + 128],
                        att2[hh][:, dj * 512 + l0 : dj * 512 + l0 + 128],
                        mask[:],
                    )
        if fillers and (len(fillers) > 1 or jp < njc // 2 - 1):
            fillers.pop(0)()
        if pend is not None:
            emit_avs(*pend)
        pend = (att2, jp)
    while len(fillers) > 1:
        fillers.pop(0)()
    emit_avs(*pend)
    # one filler held back to cover the PE while the final AV chain and
    # the avs evacuation drain (kills the ~2.4us qg-boundary bubble)
    for f in fillers:
        f()
    fillers.clear()

    # evacuate unnormalized out^T + den row straight to the bounce shards.
    # batch 0: the 512-query group spans owners 2qg, 2qg+1 (256 q each);
    # batch 1: ownership is interleaved at 128-q granularity across the
    # two half-exchanges (bounce1a = qg0/1, bounce1b = qg2/3) so the
    # first half's AllToAll fires mid-attention
    for hh in range(2):
        ob = P["ob_p"].tile([65, 512], BF16, tag="ob", name="ob")
        nc.vector.tensor_copy(ob[:], avs[hh][:])
        P["last_ob"] = ob
        if b == 0:
            bnc = P["bounce0"]
            for half in range(2):
                sh = 2 * qg + half
                nc.sync.dma_start(
                    bnc[sh * SH + hh * HOFF : sh * SH + hh * HOFF + HOFF]
                    .rearrange("(q f) -> q f", q=65),
                    ob[:, half * 256 : (half + 1) * 256],
                )
        else:
            bnc = P["bounce1a" if qg < 2 else "bounce1b"]
            for c in range(4):
                sh = (qg % 2) * 4 + c
                nc.sync.dma_start(
                    bnc[sh * SH2 + hh * HOFF2 : sh * SH2 + hh * HOFF2 + HOFF2]
                    .rearrange("(q f) -> q f", q=65),
                    ob[:, c * 128 : (c + 1) * 128],
                )


def _heartbeat(nc, P, steps, mybir):
    """Dep-chained dummy-matmul ladder: keeps the PE's HAM clock warm
    across a collective wait.  Each step is 4 N=512 matmuls (~75% PE
    duty) gated on a DVE copy of the previous step's result."""
    F32, BF16 = mybir.dt.float32, mybir.dt.bfloat16
    hb = [
        P["const_p"].tile([16, 16], BF16, tag=f"hb{i}", name=f"hb{i}")
        for i in range(2)
    ]
    nc.vector.memset(hb[0][:], 0.25)
    for k in range(steps):
        ps = P["mm_p"].tile([128, 1024], F32, tag="mm", name="hbps")
        nc.tensor.matmul(
            ps[0:16, 0:16], hb[k % 2][:], hb[k % 2][:], start=True, stop=True
        )
        # chain via the scalar engine (idle in the tail): a DVE-side chain
        # could be frozen by a hoisted collective-dependent DVE wait
        nc.scalar.copy(hb[(k + 1) % 2][:], ps[0:16, 0:16])


def _a2a(nc, P, key, mybir):
    """AllToAll the bounced shards of one exchange across all 8 cores."""
    groups = [[0, 1, 2, 3, 4, 5, 6, 7]]
    nc.gpsimd.collective_compute(
        "AllToAll", mybir.AluOpType.bypass, replica_groups=groups,
        ins=[P[f"bounce{key}"].opt()], outs=[P[f"gath{key}"].opt()],
    )


def _body(nc, P, mybir):
    F32, BF16 = mybir.dt.float32, mybir.dt.bfloat16
    _loads(nc, P, mybir)

    for b in range(2):
        qt_t = P["qk_p"].tile([128, T], BF16, tag=f"qt{b}", name=f"qt{b}")
        kt_t = P["qk_p"].tile([128, T], BF16, tag=f"kt{b}", name=f"kt{b}")
        P.setdefault("qt", []).append(qt_t)
        P.setdefault("kt", []).append(kt_t)
    # batch 0: one full exchange; batch 1: two half exchanges (1a, 1b)
    for key, W, shsz in [("0", OWN, SH), ("1a", 128, SH2), ("1b", 128, SH2)]:
        P[f"gat{key}"] = P["gat_p"].tile(
            [128, 8 * W], BF16, tag=f"gat{key}", name=f"gat{key}"
        )
        P[f"den{key}"] = P["sm_p"].tile(
            [16, W], BF16, tag=f"den{key}", name=f"den{key}"
        )
        P[f"bounce{key}"] = P["dram_p"].tile(
            [8 * shsz], BF16, tag=f"bounce{key}", name=f"bounce{key}"
        )
        P[f"gath{key}"] = P["dram_p"].tile(
            [8 * shsz], BF16, tag=f"gath{key}", name=f"gath{key}"
        )

    U = lambda *a: (lambda: _qk_unit(nc, P, *a, mybir))
    V = lambda b, *js: (lambda: _v_unit(nc, P, b, js, mybir))
    WO = lambda b, *thfs: (lambda: _wo_unit(nc, P, b, thfs, mybir))

    # lead-in: just enough q^T/k^T/v for batch-0 qg0
    _qk_unit(nc, P, 0, 1, 0, 0, mybir)
    _qk_unit(nc, P, 0, 0, 0, 0, mybir)
    _v_unit(nc, P, 0, [0, 1], mybir)

    _attention_qg(nc, P, 0, 0, [V(0, 2, 3), U(0, 1, 0, 1), U(0, 0, 0, 1)], mybir)
    _attention_qg(
        nc, P, 0, 1, [V(0, 4, 5), V(0, 6, 7), U(0, 1, 1, 0), U(0, 0, 1, 0)], mybir
    )
    _attention_qg(
        nc, P, 0, 2,
        [V(0, 8, 9), V(0, 10, 11), U(0, 1, 1, 1), U(0, 0, 1, 1),
         U(1, 1, 0, 0), U(1, 0, 0, 0)],
        mybir,
    )
    _attention_qg(
        nc, P, 0, 3,
        [V(0, 12, 13), V(0, 14, 15), V(1, 0, 1), V(1, 2, 3),
         U(1, 1, 0, 1), U(1, 0, 0, 1), U(1, 1, 1, 0), U(1, 0, 1, 0)],
        mybir,
    )
    _a2a(nc, P, "0", mybir)
    _gath_reads(nc, P, "0", mybir)

    _attention_qg(nc, P, 1, 0, [V(1, 4, 5), V(1, 6, 7)], mybir)
    _attention_qg(
        nc, P, 1, 1,
        [V(1, 8, 9), V(1, 10, 11), U(1, 1, 1, 1), U(1, 0, 1, 1)],
        mybir,
    )
    # first half-exchange fires mid-attention, fully hidden under qg2/3
    _a2a(nc, P, "1a", mybir)
    _gath_reads(nc, P, "1a", mybir)
    _attention_qg(nc, P, 1, 2, [V(1, 12, 13)], mybir)
    _attention_qg(nc, P, 1, 3, [V(1, 14, 15)], mybir)
    _a2a(nc, P, "1b", mybir)
    _gath_reads(nc, P, "1b", mybir)

    # batch-0's Wo + the first-half batch-1 Wo fill the A2A1b wait (their
    # collectives completed long ago); all normalize chains gate on qg3's
    # last evacuation so they cannot freeze the attention DVE stream
    _recip_unit(nc, P, "0", 256, P["last_ob"], mybir)
    _bc_mult_unit(nc, P, "0", 256, mybir)
    for t in range(2):
        _wo_unit(nc, P, "0", 256, 0, [(t, 0), (t, 1)], mybir)
    # each chain gates on the previous chain's output so the DVE queue
    # serves them in order (1a's data can lag the qg3 finish on slow draws)
    _recip_unit(nc, P, "1a", 128, P["last_yb"], mybir)
    _bc_mult_unit(nc, P, "1a", 128, mybir)
    _wo_unit(nc, P, "1a", 128, 256, [(0, 0), (0, 1)], mybir)
    _heartbeat(nc, P, 10, mybir)
    # tail: only the half-size A2A1b + half a Wo pass remain exposed
    _recip_unit(nc, P, "1b", 128, P["last_yb"], mybir)
    _bc_mult_unit(nc, P, "1b", 128, mybir)
    _wo_unit(nc, P, "1b", 128, 384, [(0, 0), (0, 1)], mybir)


def _build():
    import concourse.mybir as mybir
    import concourse.tile as tile
    from concourse import bacc

    F32, BF16 = mybir.dt.float32, mybir.dt.bfloat16

    nc = bacc.Bacc("TRN2", target_bir_lowering=False, debug=False, num_devices=8)
    P = {
        "xt_ext": nc.declare_dram_parameter(
            "xt", [128, 2 * CC * T], BF16, isOutput=False
        ),
        "wqkv_ext": nc.declare_dram_parameter(
            "wqkv", [128, CC * WCOL], BF16, isOutput=False
        ),
        "wo_ext": nc.declare_dram_parameter(
            "wo", [128, CC * C], BF16, isOutput=False
        ),
        "mask_ext": nc.declare_dram_parameter("mask", [128, 128], BF16, isOutput=False),
        "sel_ext": nc.declare_dram_parameter("sel", [16, 1024], BF16, isOutput=False),
        "out_ext": nc.declare_dram_parameter("out", [2 * OWN, C], F32, isOutput=True),
    }

    with tile.TileContext(nc) as tc:
        with (
            tc.tile_pool(name="const", bufs=1) as const_p,
            tc.tile_pool(name="w", bufs=1) as w_p,
            tc.tile_pool(name="x", bufs=1) as x_p,
            tc.tile_pool(name="qk", bufs=1) as qk_p,
            tc.tile_pool(name="v", bufs=1) as v_p,
            tc.tile_pool(name="att", bufs=4) as att_p,
            tc.tile_pool(name="ob", bufs=4) as ob_p,
            tc.tile_pool(name="gat", bufs=1) as gat_p,
            tc.tile_pool(name="y", bufs=2) as y_p,
            tc.tile_pool(name="sm", bufs=2) as sm_p,
            tc.tile_pool(name="mm", bufs=2, space="PSUM") as mm_p,
            tc.tile_pool(name="av", bufs=2, space="PSUM") as av_p,
            tc.tile_pool(name="aux", bufs=2, space="PSUM") as aux_p,
            tc.tile_pool(name="dram", bufs=1, space="DRAM") as dram_p,
        ):
            P.update(
                const_p=const_p, w_p=w_p, x_p=x_p, qk_p=qk_p, v_p=v_p,
                att_p=att_p, ob_p=ob_p, gat_p=gat_p, y_p=y_p, sm_p=sm_p,
                mm_p=mm_p, av_p=av_p, aux_p=aux_p, dram_p=dram_p,
            )
            _body(nc, P, mybir)

    nc.finalize()
    return nc


def kernel(x, Wqkv, bqkv, Wo, bo):
    global _cached_nc, last_result
    import ml_dtypes
    from concourse.bass_utils import run_bass_kernel_spmd

    if _cached_nc is None:
        _cached_nc = _build()
    nc = _cached_nc

    bf16 = ml_dtypes.bfloat16
    x = np.asarray(x, dtype=np.float32)
    Wqkv = np.asarray(Wqkv, dtype=np.float32)
    # Wo packed into its SBUF layout: [128, cc-chunks of 1024]
    wo_f = np.asarray(Wo, dtype=np.float32)
    wo_b = np.ascontiguousarray(
        np.concatenate(
            [wo_f[cc * 128 : (cc + 1) * 128, :] for cc in range(CC)], axis=1
        ).astype(bf16)
    )

    # x^T packed into its SBUF layout: [128, (batch, col-half, chunk) x 1024]
    xt_blocks = []
    for b in range(2):
        xb = x[b].T.astype(bf16)  # [C, T]
        for th in range(2):
            for cc in range(CC):
                xt_blocks.append(
                    xb[cc * 128 : (cc + 1) * 128, th * 1024 : (th + 1) * 1024]
                )
    xt = np.ascontiguousarray(np.concatenate(xt_blocks, axis=1))

    # lower-triangle 0/1 mask for diagonal blocks: partition = key, free = query
    tri = (np.arange(128)[:, None] <= np.arange(128)[None, :]).astype(bf16)
    tri = np.ascontiguousarray(tri)

    # selection matrix for the 1/den PE broadcast: chunk s rows 0:64 get
    # head 2s's den (sel row s), rows 64:128 get head 2s+1's (row 8+s)
    sel = np.zeros((16, 1024), dtype=bf16)
    for s in range(8):
        sel[s, s * 128 : s * 128 + 64] = 1
        sel[8 + s, s * 128 + 64 : s * 128 + 128] = 1
    sel = np.ascontiguousarray(sel)

    in_maps = []
    for core in range(8):
        c0 = core * 2 * D
        wq = Wqkv[:, c0 : c0 + 128] * SCALE
        wk = Wqkv[:, C + c0 : C + c0 + 128]
        wv = Wqkv[:, 2 * C + c0 : 2 * C + c0 + 128]
        wfull = np.concatenate([wq, wk, wv], axis=1).astype(bf16)  # [C, 384]
        # packed into SBUF layout: [128, cc-chunks of 384]
        wqkv = np.ascontiguousarray(
            np.concatenate(
                [wfull[cc * 128 : (cc + 1) * 128, :] for cc in range(CC)],
                axis=1,
            )
        )
        in_maps.append(
            {"xt": xt, "wqkv": wqkv, "wo": wo_b, "mask": tri, "sel": sel}
        )

    last_result = run_bass_kernel_spmd(nc, in_maps, core_ids=list(range(8)))

    y = np.empty((B, T, C), dtype=np.float32)
    for core in range(8):
        r = last_result.results[core]["out"]
        y[0, core * OWN : (core + 1) * OWN, :] = r[0:OWN]
        # batch 1 ownership is interleaved at 128 rows per half-exchange
        y[1, core * 128 : (core + 1) * 128, :] = r[256:384]
        y[1, 1024 + core * 128 : 1024 + (core + 1) * 128, :] = r[384:512]
    return y
